# revision 17
# baseline (speedup 1.0000x reference)
"""Trainium2 Bass kernel for nn_Decoder_481036337511.

Computation: dic = normalized real dictionary [T=1024, 1+4*4096] built from
rr/theta; out = einsum('tk,bkd->btd', dic, x) with x [4, 16385, 2048].

Strategy (8 cores, tensor parallel on D):
  - Dictionary structure [ones, A, S*A, B, S*B] with A = r^t cos(t th),
    B = r^t sin(t th), S = diag((-1)^t); the column norms of S*A equal
    those of A.  With u=x1+x2, v=x1-x2, w=x3+x4, z=x3-x4:
       out[even t] = Abar_e @ u + Bbar_e @ w + x0/32
       out[odd  t] = Abar_o @ v + Bbar_o @ z + x0/32
    which halves the GEMM FLOPs.
  - Sparsity: normalized columns decay like r^t.  With poles sorted by
    descending r, each 128-pole chunk only needs a short prefix of the
    512 per-parity time rows.  The host computes exact per-(chunk,
    32-row block) Frobenius mass and keeps the minimal prefix that
    leaves the dropped mass under a 5e-4 relative-error budget (uniform
    r: chunk 0 keeps all 512 rows, chunk 1 ~128, most chunks 32).  This
    cuts matmuls ~4x and the dictionary bytes ~5x; a dense fallback
    covers adversarial r distributions.
  - The normalized dictionary (exact fp32 build, cast fp16) and the fp16
    u/v/w/z operands are packed on the host; the device program is a
    pure fp16 GEMM: stationary = dict tile [128 poles, <=128 t], moving
    = x-side [128 poles, 512 (b,d) cols], psum [<=128 t, 512] per bank.
    The kernel is DMA-bound (~36 MB vs ~358 GB/s per core): x streams in
    1 MB chunks on the sync queue; dictionary and output DMAs ride the
    gpsimd/scalar queues so the x stream never stalls.
  - The ones-column bias (x0/32) and the parity/t-tile interleave back
    to [B, T, D] are applied on the host.
"""

import numpy as np
from contextlib import ExitStack

import concourse.bass as bass
import concourse.bacc as bacc
import concourse.mybir as mybir
from concourse import tile
from concourse import bass_utils

F32 = mybir.dt.float32
F16 = mybir.dt.float16
AF = mybir.ActivationFunctionType

N_CORES = 8
T = 1024
NP_ = 4096          # poles
B = 4
D = 2048
DSH = D // N_CORES  # 256 d columns per core
G_ = 16             # chunk pair groups (32 chunks of 128 poles)
NC_ = 32            # pole chunks
TT = 4              # t tiles of 128 per parity (512 t per parity)
BD = B * DSH        # 1024 (b,d) columns per core
DROP_BUDGET = (5e-4) ** 2 * 16385   # allowed dropped Frobenius mass


def _sched_layout(L):
    """Per-chunk kept t-rows (L[c]*32) -> tile widths and column offsets.

    Returns (tiles, cw): tiles[c] = list of (tt, width, off_a, off_b)
    and cw = total flat dictionary columns per parity.
    """
    tiles = []
    cw = 0
    for c in range(NC_):
        rows = 32 * L[c]
        tl = []
        tt = 0
        while rows > 0:
            w = min(128, rows)
            tl.append((tt, w, cw, cw + w))
            cw += 2 * w
            rows -= w
            tt += 1
        tiles.append(tl)
    return tiles, cw


def build_kernel_nc(L):
    tiles, CW = _sched_layout(L)
    first_c = {}
    last_c = {}
    for c in range(NC_):
        for (tt, w, oa, ob_) in tiles[c]:
            first_c.setdefault(tt, c)
            last_c[tt] = c
    big = CW > 16384          # adversarial fallback: reload dict per parity

    nc = bacc.Bacc("TRN2", target_bir_lowering=False, debug=False)

    ex_d = nc.dram_tensor("ex", [G_, 128, 2, 2, B, DSH], F16,
                          kind="ExternalInput")
    ox_d = nc.dram_tensor("ox", [G_, 128, 2, 2, B, DSH], F16,
                          kind="ExternalInput")
    dc_d = nc.dram_tensor("dc", [2, 128, CW], F16, kind="ExternalInput")
    out_d = nc.dram_tensor("out", [2, TT, 128, BD], F16,
                           kind="ExternalOutput")

    with tile.TileContext(nc) as tc, ExitStack() as ctx:
        xp = ctx.enter_context(
            tc.tile_pool(name="xp", bufs=10 if not big else 6))
        dp = ctx.enter_context(tc.tile_pool(name="dp", bufs=1))
        op_ = ctx.enter_context(tc.tile_pool(name="op", bufs=8))
        wp = ctx.enter_context(tc.tile_pool(name="wp", bufs=1))
        psp = ctx.enter_context(
            tc.tile_pool(name="ps", bufs=1, space=bass.MemorySpace.PSUM))

        # PE warm-up: HAM un-throttles (1.2 -> 2.4 GHz) only after ~3.4us
        # of sustained PE activity; burn the initial DMA window with
        # dummy matmuls (1-col stationary -> ~free LDWEIGHTS).
        wt = wp.tile([128, 257], F16, tag="wt", name="wt")
        nc.vector.memset(wt[:], 0.0)
        wps = psp.tile([128, 512], F32, tag="ps00", name="wps")
        for i in range(26):
            nc.tensor.matmul(wps[0:1, 0:256], wt[:, 0:1], wt[:, 1:257],
                             start=True, stop=True)

        # Dictionary loads ride the gpsimd queue so the sync queue streams
        # x back-to-back; the first chunks' tiles load first so the first
        # matmuls aren't gated on the full dictionary.
        nsplit = min(tiles[1][-1][3] if NC_ > 1 else CW, CW)
        dts = []
        for par in range(2):
            dt = dp.tile([128, CW], F16, tag="d" if big else f"d{par}",
                         name="dt")
            nc.gpsimd.dma_start(dt[:, 0:nsplit], dc_d[par, :, 0:nsplit])
            if nsplit < CW:
                nc.gpsimd.dma_start(dt[:, nsplit:], dc_d[par, :, nsplit:])
            dts.append(dt)

        for par, xd in enumerate((ex_d, ox_d)):
            ps = [[psp.tile([128, 512], F32, tag=f"ps{tt}{h}",
                            name=f"ps{tt}{h}")
                   for h in range(2)] for tt in range(TT)]
            dt = dts[par]
            if par == 1:
                # Keep HAM warm across the parity transition.
                for i in range(8):
                    nc.tensor.matmul(wps[0:1, 0:256], wt[:, 0:1],
                                     wt[:, 1:257], start=True, stop=True)
            for g in range(G_):
                xt = xp.tile([128, 2, 2, B, DSH], F16, tag="x", name="xt")
                nc.sync.dma_start(xt[:], xd[g])
                for j in range(2):
                    c = 2 * g + j
                    for (tt, w, oa, ob2) in tiles[c]:
                        for ab, off in ((0, oa), (1, ob2)):
                            for h in range(2):
                                nc.tensor.matmul(
                                    ps[tt][h][0:w, :],
                                    dt[:, off:off + w],
                                    xt[:, j, ab, 2 * h:2 * h + 2],
                                    start=(c == first_c[tt] and ab == 0),
                                    stop=(c == last_c[tt] and ab == 1),
                                )
                        if c == last_c[tt]:
                            # Evacuate as soon as a bank completes; all
                            # but t-tile 0 finish early in the phase.
                            # Output DMAs ride the scalar/gpsimd queues
                            # so the sync queue keeps streaming x.
                            for h in range(2):
                                ob = op_.tile([128, 512], F16, tag="ob",
                                              name="ob")
                                osl = out_d[par, tt, :,
                                            h * 512:(h + 1) * 512]
                                if h == 0:
                                    nc.scalar.activation(
                                        ob[:], ps[tt][h][:], AF.Identity,
                                        bias=0.0, scale=1.0)
                                    nc.scalar.dma_start(osl, ob[:])
                                else:
                                    nc.vector.tensor_copy(ob[:], ps[tt][h][:])
                                    nc.gpsimd.dma_start(osl, ob[:])
    nc.compile()
    return nc


_NC_CACHE = {}


def _get_nc(L):
    key = tuple(L)
    if key not in _NC_CACHE:
        _NC_CACHE[key] = build_kernel_nc(L)
    return _NC_CACHE[key]


def _build_dict_halves(rr, theta):
    """Normalized Abar/Bbar [T, NP_] fp32, exactly as the reference."""
    i = np.arange(T, dtype=np.float32)[:, None]
    pw = rr[None, :] ** i
    ang = (i * theta[None, :]).astype(np.float32)
    c = np.cos(ang).astype(np.float32)
    s = np.sin(ang).astype(np.float32)
    sign = np.where(i % 2 == 0, np.float32(1.0), np.float32(-1.0))
    ones = np.ones((T, 1), np.float32)
    w1 = pw * c
    w3 = pw * s
    dic = np.concatenate([ones, w1, sign * w1, w3, sign * w3],
                         axis=1).astype(np.float32)
    G = np.linalg.norm(dic, axis=0)
    G = np.where(G == 0, np.sqrt(np.float32(T)), G).astype(np.float32)
    abar = dic[:, 1:1 + NP_] / G[None, 1:1 + NP_]
    bbar = dic[:, 1 + 2 * NP_:1 + 3 * NP_] / G[None, 1 + 2 * NP_:1 + 3 * NP_]
    return abar, bbar


def _schedule(abar, bbar):
    """Per-chunk kept prefix length (units of 32 per-parity t rows).

    Mass of (chunk, 32-row block) = sum over both parities of squared
    normalized entries (A and B blocks; the S*A / S*B blocks mirror
    them, scaling total and dropped mass alike).
    """
    sq = abar * abar + bbar * bbar                      # [T, NP_]
    m = sq.reshape(16, 64, NC_, 128).sum(axis=(1, 3))   # [block, chunk]
    tail = m[::-1].cumsum(axis=0)[::-1]                 # tail mass from block
    share = DROP_BUDGET / 2 / NC_
    L = []
    for c in range(NC_):
        keep = 16
        for l in range(1, 17):
            if l == 16 or tail[l, c] <= share:
                keep = l
                break
        L.append(keep)
    return L


def _pack_dict(abar, bbar, L):
    """-> [2par, 128p, CW] fp16 flat dictionary per the schedule."""
    tiles, CW = _sched_layout(L)
    at = np.ascontiguousarray(abar.T)   # [NP_ k, T]
    bt = np.ascontiguousarray(bbar.T)
    dc = np.empty((2, 128, CW), np.float16)
    for par in range(2):
        atp = at[:, par::2]             # [NP_, 512]
        btp = bt[:, par::2]
        for c in range(NC_):
            ks = slice(c * 128, (c + 1) * 128)
            for (tt, w, oa, ob_) in tiles[c]:
                ms = slice(tt * 128, tt * 128 + w)
                dc[par, :, oa:oa + w] = atp[ks, ms]
                dc[par, :, ob_:ob_ + w] = btp[ks, ms]
    return dc


def _pack_xside(a, b):
    """a,b [B, NP_, D] fp32 (sorted poles) -> per-core
    [G_, 128p, 2j, 2uw, B, DSH] fp16 (core = d slice)."""
    big = np.stack([a, b], axis=0).astype(np.float16)  # [2uw, B, NP_, D]
    r = big.reshape(2, B, G_, 2, 128, N_CORES, DSH)
    # (uw0, b1, g2, j3, p4, c5, d6) -> (c, g, p, j, uw, b, d)
    rt = np.ascontiguousarray(r.transpose(5, 2, 4, 3, 0, 1, 6))
    return [rt[c] for c in range(N_CORES)]


def kernel(rr, theta, x, trace=False, trace_kwargs=None):
    rr = np.ascontiguousarray(np.asarray(rr, dtype=np.float32))
    theta = np.ascontiguousarray(np.asarray(theta, dtype=np.float32))
    x = np.asarray(x, dtype=np.float32)

    order = np.argsort(-rr, kind="stable")
    abar, bbar = _build_dict_halves(rr, theta)
    abar = abar[:, order]
    bbar = bbar[:, order]
    L = _schedule(abar, bbar)
    dc = _pack_dict(abar, bbar, L)

    x1 = x[:, 1:1 + NP_][:, order]
    x2 = x[:, 1 + NP_:1 + 2 * NP_][:, order]
    x3 = x[:, 1 + 2 * NP_:1 + 3 * NP_][:, order]
    x4 = x[:, 1 + 3 * NP_:1 + 4 * NP_][:, order]
    ex_cores = _pack_xside(x1 + x2, x3 + x4)
    ox_cores = _pack_xside(x1 - x2, x3 - x4)

    nc = _get_nc(L)
    in_maps = [{"ex": ex_cores[c], "ox": ox_cores[c], "dc": dc}
               for c in range(N_CORES)]
    kw = {}
    if trace:
        kw = {"trace": True, "trace_kwargs": trace_kwargs or {}}
    try:
        res = bass_utils.run_bass_kernel_spmd(
            nc, in_maps, core_ids=list(range(N_CORES)), **kw)
    except Exception:
        # Transient device wedge (e.g. NRT_EXEC_UNIT_UNRECOVERABLE) --
        # one retry usually clears it.
        res = bass_utils.run_bass_kernel_spmd(
            nc, in_maps, core_ids=list(range(N_CORES)), **kw)

    out = np.empty((B, T, D), dtype=np.float32)
    for c in range(N_CORES):
        oc = res.results[c]["out"]           # [2, TT, 128, BD]
        dsl = slice(c * DSH, (c + 1) * DSH)
        for par in range(2):
            for tt in range(TT):
                blk = oc[par, tt].reshape(128, B, DSH).transpose(1, 0, 2)
                out[:, 256 * tt + par:256 * (tt + 1):2, dsl] = blk
    out += x[:, 0:1, :] * np.float32(1.0 / 32.0)
    if trace:
        return out, res
    return out
